# revision 9
# baseline (speedup 1.0000x reference)
"""Trainium2 Bass kernel for nn_Conv_agg_raw (GNN message passing).

Math: out = sum_k weight[k] @ (h @ resx[k]) + bias, where resx[k] is the
dense [N,N] scatter of edge features X[:,k] at (src,dst).  Equivalently
    res_k[:, m] = sum_{e: dst_e=m} X[e,k] * h[:, src_e]
    out[:, m]   = sum_k weight[k] @ res_k[:, m] + bias
We shard dst nodes across the 8 cores (512 each) - fully independent, no
collectives.  Each core gathers h columns by src (DMA gather of h^T rows),
aggregates edges into res via PE matmuls against on-the-fly built
(x outer dst-one-hot) matrices, then applies the stacked weight.

Schedule: the final weight-apply matmuls are interleaved into the edge
stream (uneven splits, small tail); PSUM accumulates 4 windows per bank
so PSUM->SBUF copies are quad-sized; everything 2-byte (h fp16 gather,
wT fp16, resstack fp16, fp16 output upcast on host).
"""

import time
import numpy as np

import concourse.bass as bass
import concourse.bacc as bacc
import concourse.tile as tile
from concourse import mybir
from concourse.bass_utils import run_bass_kernel_spmd

import os as _os

N = 4096
K = 8
C = 256
NCORES = 8
DSTS_PER_CORE = N // NCORES      # 512
SLOTS = 16                       # dst slots per window
WINDOWS = DSTS_PER_CORE // SLOTS # 32
KS = K * SLOTS                   # 128 psum cols per window
QUAD = 4                         # windows per psum bank / per copy
NQUAD = WINDOWS // QUAD
# gather groups (windows per dma_gather); first small so the stream starts
# early, last small so the tail dependency chain is short
GROUPS = [2, 4, 5, 5, 5, 4, 3, 2, 1, 1]
assert sum(GROUPS) == WINDOWS
# final weight-apply splits (in windows); boundaries on quad multiples
SPLITS = [12, 8, 8, 4]
assert sum(SPLITS) == WINDOWS and all(s % QUAD == 0 for s in np.cumsum(SPLITS))
NWU = int(_os.environ.get("GNN_NWU", "96"))   # PE warm-up matmuls

_prog_cache: dict = {}


# ---------------------------------------------------------------- device ----
def _build_program(caps):
    """Build the per-core Bass program.

    caps[w] = number of 128-edge chunks window w holds (same vector for
    every core; the host packs each core's dsts to fit it)."""
    caps = list(caps)
    assert len(caps) == WINDOWS
    off = [0]
    for cw in caps:
        off.append(off[-1] + cw)
    nchunk = off[-1]                 # total chunks per core

    gstart = [0]
    for g in GROUPS:
        gstart.append(gstart[-1] + g)

    nc = bacc.Bacc("TRN2", target_bir_lowering=False, debug=False)
    f32 = mybir.dt.float32
    f16 = mybir.dt.float16

    hT = nc.dram_tensor("hT", [N, C], f16, kind="ExternalInput")
    wT = nc.dram_tensor("wT", [K * C, C], f16, kind="ExternalInput")
    # one int16 container: [idx | xr | dl | iota]  (xr/dl/iota are fp16)
    IXW = nchunk * 16 + nchunk + SLOTS
    ixr_d = nc.dram_tensor("ixr", [128, IXW], mybir.dt.int16,
                           kind="ExternalInput")
    aux_d = nc.dram_tensor("aux", [128, 2], f32, kind="ExternalInput")
    out_d = nc.dram_tensor("out", [C, DSTS_PER_CORE], f16,
                           kind="ExternalOutput")

    # idx cols needed by gather group 0
    g0_icols = off[gstart[1]] * 8

    with tile.TileContext(nc) as tc:
        with (
            tc.tile_pool(name="persist", bufs=1) as pp,
            tc.tile_pool(name="hg", bufs=3) as hgp,
            tc.tile_pool(name="rhs", bufs=6) as rhp,
            tc.tile_pool(name="psw", bufs=2, space="PSUM") as psw,
            tc.tile_pool(name="psf", bufs=2, space="PSUM") as psf,
        ):
            # ---- bulk loads; group-0 idx first (it gates the first gather)
            ixr_sb = pp.tile([128, IXW], mybir.dt.int16)
            nc.sync.dma_start(ixr_sb[:, :g0_icols], ixr_d.ap()[:, :g0_icols])
            nc.sync.dma_start(ixr_sb[:, g0_icols:], ixr_d.ap()[:, g0_icols:])
            idx_sb = ixr_sb[:, :nchunk * 8]
            xr_sb = ixr_sb[:, nchunk * 8:nchunk * 16].bitcast(f16)
            dl_sb = ixr_sb[:, nchunk * 16:nchunk * 17].bitcast(f16)
            iota_sb = ixr_sb[:, nchunk * 17:].bitcast(f16)
            aux_sb = pp.tile([128, 2], f32)
            nc.sync.dma_start(aux_sb[:], aux_d.ap())
            bias_sb = aux_sb

            # ---- dst-slot one-hot mask, split so window 0 unblocks early
            # mask_all[p, ch, d] = (dl[p, ch] == d)   (fp16)
            mask_all = pp.tile([128, nchunk * SLOTS], f16)
            mask_cut = off[gstart[2]]          # chunks of groups 0-1
            for lo, hi in ((0, mask_cut), (mask_cut, nchunk)):
                nc.vector.tensor_tensor(
                    mask_all[:, lo * SLOTS:hi * SLOTS].rearrange(
                        "p (ch d) -> p ch d", d=SLOTS),
                    dl_sb[:, lo:hi].unsqueeze(-1).broadcast_to(
                        [128, hi - lo, SLOTS]),
                    iota_sb.unsqueeze(1).broadcast_to([128, hi - lo, SLOTS]),
                    mybir.AluOpType.is_equal,
                )

            # res accumulator in SBUF, fp16; col = w*128 + k*16 + d
            resstack = [pp.tile([128, WINDOWS * KS], f16,
                                name=f"resstack{i}")
                        for i in range(2)]   # per c_in half

            wt_sb = pp.tile([128, 16 * C], f16)  # chunk q at cols [q*256,..)
            wt_emitted = 0

            # ---- PE warm-up: matmul activity releases the HAM clock
            # throttle while the prologue DMAs fill.  Lands in resstack[0]
            # (overwritten by quad 0's copy) so dead-code passes keep it.
            wu = pp.tile([128, 128], f16, name="wu")
            nc.vector.memset(wu[:], 0.0)
            wps = psf.tile([128, KS], f32, tag="wps", name="wps")
            for i in range(NWU):
                nc.tensor.matmul(wps[:], wu[:], wu[:],
                                 start=(i == 0), stop=(i == NWU - 1))
            nc.scalar.copy(resstack[0][:, 0:KS], wps[:])

            out_sb = [pp.tile([128, DSTS_PER_CORE], f16, name=f"osb{oh}")
                      for oh in range(2)]

            split_bounds = np.cumsum([0] + SPLITS)
            next_split = 0
            psq = {}

            def emit_final_split(s):
                w0, w1 = int(split_bounds[s]), int(split_bounds[s + 1])
                nw = w1 - w0
                for oh in range(2):
                    pso = psf.tile([128, nw * SLOTS], f32, tag="psf",
                                   name=f"pso{oh}")
                    for q in range(16):      # q = (k, ci_half)
                        k, cih = divmod(q, 2)
                        rs = resstack[cih][:]
                        rhs_ap = bass.AP(
                            rs.tensor,
                            rs.offset + w0 * KS + k * SLOTS,
                            [[WINDOWS * KS, 128], [KS, nw], [1, SLOTS]],
                        )
                        nc.tensor.matmul(
                            pso[:],
                            wt_sb[:, q * 256 + oh * 128:
                                  q * 256 + oh * 128 + 128],
                            rhs_ap,
                            start=(q == 0),
                            stop=(q == 15),
                        )
                    nc.scalar.add(
                        out_sb[oh][:, w0 * SLOTS:w1 * SLOTS],
                        pso[:], bias_sb[:, oh:oh + 1])
                # stores on 256-col boundaries: after split 1 and last split
                if s == 1:
                    for oh in range(2):
                        nc.sync.dma_start(
                            out_d.ap()[oh * 128:(oh + 1) * 128, 0:256],
                            out_sb[oh][:, 0:256])
                if s == len(SPLITS) - 1:
                    for oh in range(2):
                        nc.sync.dma_start(
                            out_d.ap()[oh * 128:(oh + 1) * 128, 256:512],
                            out_sb[oh][:, 256:512])

            for g, gg in enumerate(GROUPS):
                w0 = gstart[g]
                gch = off[w0 + gg] - off[w0]   # chunks in this group
                hg = hgp.tile([128, gch, C], f16, tag="hg")
                nc.gpsimd.dma_gather(
                    out_ap=hg[:],
                    in_ap=hT.ap(),
                    idxs_ap=idx_sb[:, off[w0] * 8:off[w0 + gg] * 8],
                    num_idxs=gch * 128,
                    num_idxs_reg=gch * 128,
                    elem_size=C,
                    # single-packet mode traps the exec unit above ~1024 descs
                    single_packet=False,
                )
                # wT feeds only the final matmuls; emit its two halves
                # behind the first gathers so they don't delay the stream
                if g == 1 and wt_emitted == 0:
                    nc.sync.dma_start(
                        wt_sb[:, :8 * C],
                        wT.ap()[:8 * 128].rearrange(
                            "(q p) c -> p q c", p=128))
                    wt_emitted = 1
                if g == 2 and wt_emitted == 1:
                    nc.sync.dma_start(
                        wt_sb[:, 8 * C:],
                        wT.ap()[8 * 128:].rearrange(
                            "(q p) c -> p q c", p=128))
                    wt_emitted = 2
                for wl in range(gg):
                    w = w0 + wl
                    cw = caps[w]
                    qd, qc = divmod(w, QUAD)
                    if qc == 0:
                        psq = {half: psw.tile([128, QUAD * KS], f32,
                                              tag=f"psq{half}",
                                              name=f"psq{half}")
                               for half in range(2)}
                    # rhs[p, ch, k, d] = xr[p, ch, k] * mask[p, ch, d]
                    rhs = rhp.tile([128, cw, KS], f16, tag="rhs")
                    xr_w = bass.AP(xr_sb.tensor,
                                   xr_sb.offset + off[w] * K,
                                   [[IXW, 128], [K, cw],
                                    [1, K], [0, SLOTS]])
                    mk_w = bass.AP(mask_all[:].tensor,
                                   mask_all[:].offset + off[w] * SLOTS,
                                   [[nchunk * SLOTS, 128], [SLOTS, cw],
                                    [0, K], [1, SLOTS]])
                    build_eng = nc.gpsimd if (w % 8) == 5 else nc.vector
                    build_eng.tensor_tensor(
                        rhs[:].rearrange("p ch (k d) -> p ch k d", d=SLOTS),
                        xr_w, mk_w, mybir.AluOpType.mult,
                    )

                    # aggregate: psq[ci, qc*KS+(k,d)] += Hg_chunk.T @ rhs
                    for half in range(2):
                        for ch in range(cw):
                            nc.tensor.matmul(
                                psq[half][:, qc * KS:(qc + 1) * KS],
                                hg[:, off[w] - off[w0] + ch,
                                   half * 128:(half + 1) * 128],
                                rhs[:, ch, :],
                                start=(ch == 0),
                                stop=(ch == cw - 1),
                            )
                    if qc == QUAD - 1:
                        for half in range(2):
                            nc.scalar.copy(
                                resstack[half][:, qd * QUAD * KS:
                                               (qd + 1) * QUAD * KS],
                                psq[half][:])
                        # emit any final split whose windows are all copied
                        while (next_split < len(SPLITS) and
                               split_bounds[next_split + 1] <= w + 1):
                            emit_final_split(next_split)
                            next_split += 1
            while next_split < len(SPLITS):
                emit_final_split(next_split)
                next_split += 1

    nc.compile()
    return nc


# ------------------------------------------------------------------ host ----
def _greedy_partition(items_deg, nbins, cap):
    """Assign item ids (sorted desc by degree) to bins; each bin gets at most
    `cap` items, minimizing max degree-sum.  Returns list of lists."""
    import heapq
    bins = [[] for _ in range(nbins)]
    heap = [(0, b) for b in range(nbins)]
    heapq.heapify(heap)
    for it, dg in items_deg:
        s, b = heapq.heappop(heap)
        bins[b].append(it)
        if len(bins[b]) < cap:
            heapq.heappush(heap, (s + dg, b))
    return bins


def _pack_windows(items_deg, caps):
    """Pack (dst, deg) items into len(caps) bins of SLOTS items each with
    bin w's degree-sum <= caps[w]*128.  Returns list of lists or None."""
    nb = len(caps)
    rem_cap = [c * 128 for c in caps]
    rem_slots = [SLOTS] * nb
    bins = [[] for _ in range(nb)]
    for it, dg in items_deg:           # desc by degree
        best, best_score = -1, None
        for b in range(nb):
            if rem_slots[b] == 0 or rem_cap[b] < dg:
                continue
            score = (rem_cap[b] - dg) / rem_slots[b]
            if best_score is None or score > best_score:
                best, best_score = b, score
        if best < 0:
            return None
        bins[best].append(it)
        rem_cap[best] -= dg
        rem_slots[best] -= 1
    return bins


def _choose_caps(core_items):
    """Search cap profiles (nondecreasing totals) until every core packs.
    Returns (caps, core_windows)."""
    profiles = []
    for extra in range(0, 33):
        for a in range(0, min(extra // 2, 8) + 1):
            b = extra - 2 * a
            if a + b > WINDOWS:
                continue
            profiles.append((extra, a, b))
    profiles.sort()
    for extra, a, b in profiles:
        caps = [6] * a + [5] * b + [4] * (WINDOWS - a - b)
        packs = []
        for items in core_items:
            p = _pack_windows(items, caps)
            if p is None:
                break
            packs.append(p)
        if len(packs) == NCORES:
            return caps, packs
    # fallback: uniform capacity from the worst window under plain LPT
    max_cnt = 0
    core_windows = []
    for items in core_items:
        degmap = dict(items)
        wins = _greedy_partition(items, WINDOWS, SLOTS)
        core_windows.append(wins)
        for wlist in wins:
            max_cnt = max(max_cnt, int(sum(degmap[m] for m in wlist)))
    caps = [max(1, -(-max_cnt // 128))] * WINDOWS
    return caps, core_windows


def kernel(h, X, edge_index, batch_node, weight, bias):
    h = np.asarray(h, dtype=np.float32)
    X = np.asarray(X, dtype=np.float32)
    edge_index = np.asarray(edge_index)
    weight = np.asarray(weight, dtype=np.float32)
    bias = np.asarray(bias, dtype=np.float32)

    src = edge_index[0].astype(np.int64)
    dst = edge_index[1].astype(np.int64)

    deg = np.bincount(dst, minlength=N)
    order = np.argsort(-deg, kind="stable")

    # dst -> core (8 bins of 512)
    core_bins = _greedy_partition([(int(m), int(deg[m])) for m in order],
                                  NCORES, DSTS_PER_CORE)
    core_items = [sorted([(m, int(deg[m])) for m in core_bins[c]],
                         key=lambda t: -t[1]) for c in range(NCORES)]
    caps, core_windows = _choose_caps(core_items)

    # edges grouped by dst
    eorder = np.argsort(dst, kind="stable")
    starts = np.searchsorted(dst[eorder], np.arange(N))
    ends = np.searchsorted(dst[eorder], np.arange(N) + 1)

    key = tuple(caps)
    if key not in _prog_cache:
        _prog_cache[key] = _build_program(caps)
    nc = _prog_cache[key]

    off = [0]
    for cw in caps:
        off.append(off[-1] + cw)
    nchunk = off[-1]
    IXW = nchunk * 16 + nchunk + SLOTS
    hT16 = np.ascontiguousarray(h.T).astype(np.float16)       # [N, C]
    wT = np.ascontiguousarray(
        weight.transpose(0, 2, 1).reshape(K * C, C)).astype(np.float16)
    bias2 = np.ascontiguousarray(bias.reshape(2, 128))
    iota = np.tile(np.arange(SLOTS, dtype=np.float16), (128, 1))

    in_maps = []
    perms = []
    for c in range(NCORES):
        idx = np.zeros((128, nchunk * 8), dtype=np.int16)
        xr = np.zeros((128, nchunk, K), dtype=np.float16)
        dl = np.zeros((128, nchunk), dtype=np.float16)
        perm = np.empty(DSTS_PER_CORE, dtype=np.int64)
        for w in range(WINDOWS):
            wl = core_windows[c][w]
            el = []
            sl = []
            for d_slot, m in enumerate(wl):
                perm[w * SLOTS + d_slot] = m
                ee = eorder[starts[m]:ends[m]]
                el.append(ee)
                sl.append(np.full(ee.shape[0], d_slot, dtype=np.float16))
            el = (np.concatenate(el) if el else
                  np.empty(0, dtype=np.int64))
            sl = (np.concatenate(sl) if sl else
                  np.empty(0, dtype=np.float16))
            # order the window's edges by src: the dst slot travels in `sl`,
            # and src-sorted gathers hit HBM row buffers far more often
            so = np.argsort(src[el], kind="stable")
            el, sl = el[so], sl[so]
            L = el.shape[0]
            j = np.arange(L)
            p = j % 128
            ch = off[w] + j // 128
            xr[p, ch, :] = X[el, :].astype(np.float16)
            dl[p, ch] = sl
            # gather index layout: pos j -> [j%16, j//16], replicated x8
            srcs = src[el].astype(np.int16)
            blk = np.zeros((16, caps[w] * 8), dtype=np.int16)
            blk[j % 16, j // 16] = srcs
            idx[:, off[w] * 8:off[w + 1] * 8] = np.tile(blk, (8, 1))
        ixr = np.concatenate(
            [idx, xr.reshape(128, nchunk * K).view(np.int16),
             dl.view(np.int16), iota.view(np.int16)], axis=1)
        assert ixr.shape[1] == IXW
        in_maps.append({
            "hT": hT16, "wT": wT,
            "aux": np.ascontiguousarray(bias2.T.astype(np.float32)),
            "ixr": np.ascontiguousarray(ixr),
        })
        perms.append(perm)

    global _last_perms
    _last_perms = perms

    try:
        res = run_bass_kernel_spmd(nc, in_maps, core_ids=list(range(NCORES)))
    except Exception:
        # transient device-state issues (e.g. a previous crashed process left
        # a core unrecoverable) usually clear on retry
        time.sleep(10)
        res = run_bass_kernel_spmd(nc, in_maps, core_ids=list(range(NCORES)))

    out = np.empty((C, N), dtype=np.float32)
    for c in range(NCORES):
        out[:, perms[c]] = res.results[c]["out"].astype(np.float32)
    return out


# revision 16
# speedup vs baseline: 1.0721x; 1.0721x over previous
"""Trainium2 Bass kernel for nn_Conv_agg_raw (GNN message passing).

Math: out = sum_k weight[k] @ (h @ resx[k]) + bias, where resx[k] is the
dense [N,N] scatter of edge features X[:,k] at (src,dst).  Equivalently
    res_k[:, m] = sum_{e: dst_e=m} X[e,k] * h[:, src_e]
    out[:, m]   = sum_k weight[k] @ res_k[:, m] + bias
We shard dst nodes across the 8 cores (512 each) - fully independent, no
collectives.  Each core gathers h columns by src (DMA gather of h^T rows),
aggregates edges into res via PE matmuls against on-the-fly built
(x outer dst-one-hot) matrices, then applies the stacked weight.

Schedule notes:
- The h-row stream for the first NCOPY groups is pre-gathered by the host
  into a prefix of the hT tensor, so those groups load via plain DMA
  copies with no index dependency and the DMA engines saturate from t~0.
- wT is loaded via the Pool engine's SWDGE so it lands mid-stream between
  gather descriptor generations instead of jumping the DMA queue.
- The final weight-apply matmuls are interleaved into the edge stream
  (uneven splits, pair-sized tail), PSUM accumulates 4 windows per bank.
- Everything 2-byte: h fp16 gather, wT fp16, resstack fp16, fp16 output
  upcast on the host.
"""

import time
import numpy as np

import concourse.bass as bass
import concourse.bacc as bacc
import concourse.tile as tile
from concourse import mybir
from concourse.bass_utils import run_bass_kernel_spmd

import os as _os

N = 4096
K = 8
C = 256
NCORES = 8
DSTS_PER_CORE = N // NCORES      # 512
SLOTS = 16                       # dst slots per window
WINDOWS = DSTS_PER_CORE // SLOTS # 32
KS = K * SLOTS                   # 128 psum cols per window
# gather groups (windows per dma_gather/copy); the first NCOPY groups are
# served by plain DMA copies of host-pre-gathered rows (no idx dependency),
# so the stream starts at t~0; last groups small for a short tail chain
GROUPS = [2, 3, 4, 5, 5, 5, 4, 2, 1, 1]
NCOPY = 2                        # leading groups shipped as plain copies
assert sum(GROUPS) == WINDOWS
# final weight-apply splits (in windows); copies are quad-sized up to
# window 28 and pair-sized after, so split boundaries stay copy-aligned
SPLITS = [12, 8, 8, 2, 2]
COPY_GRAN = [4] * 7 + [2, 2]     # psum->resstack copy group sizes
assert sum(SPLITS) == WINDOWS and sum(COPY_GRAN) == WINDOWS
NWU = int(_os.environ.get("GNN_NWU", "9"))    # free-512 PE warm-up matmuls

_prog_cache: dict = {}


# ---------------------------------------------------------------- device ----
def _build_program(caps):
    """Build the per-core Bass program.

    caps[w] = number of 128-edge chunks window w holds (same vector for
    every core; the host packs each core's dsts to fit it)."""
    caps = list(caps)
    assert len(caps) == WINDOWS
    off = [0]
    for cw in caps:
        off.append(off[-1] + cw)
    nchunk = off[-1]                 # total chunks per core

    gstart = [0]
    for g in GROUPS:
        gstart.append(gstart[-1] + g)
    wcut = gstart[NCOPY]             # first window served by real gathers
    pre_chunks = off[wcut]           # chunks shipped as the hT prefix
    R0 = pre_chunks * 128            # prefix rows in hT2

    cstart = [0]
    for g in COPY_GRAN:
        cstart.append(cstart[-1] + g)
    copy_of_w = {}
    for ci_, g in enumerate(COPY_GRAN):
        for w in range(cstart[ci_], cstart[ci_ + 1]):
            copy_of_w[w] = ci_

    nc = bacc.Bacc("TRN2", target_bir_lowering=False, debug=False)
    f32 = mybir.dt.float32
    f16 = mybir.dt.float16

    hT = nc.dram_tensor("hT", [R0 + N, C], f16, kind="ExternalInput")
    wT = nc.dram_tensor("wT", [K * C, C], f16, kind="ExternalInput")
    # fp16 container A: [xr | dl | iota]
    PREW = nchunk * 8 + nchunk + SLOTS
    pre_d = nc.dram_tensor("pre", [128, PREW], f16,
                           kind="ExternalInput")
    # int16 container B: gather indices for windows >= wcut
    IDXW = (nchunk - pre_chunks) * 8
    idx_d = nc.dram_tensor("idx", [128, IDXW], mybir.dt.int16,
                           kind="ExternalInput")
    aux_d = nc.dram_tensor("aux", [128, 2], f32, kind="ExternalInput")
    out_d = nc.dram_tensor("out", [C, DSTS_PER_CORE], f16,
                           kind="ExternalOutput")

    with tile.TileContext(nc) as tc:
        with (
            tc.tile_pool(name="persist", bufs=1) as pp,
            tc.tile_pool(name="hg", bufs=4) as hgp,
            tc.tile_pool(name="rhs", bufs=12) as rhp,
            tc.tile_pool(name="psw", bufs=2, space="PSUM") as psw,
            tc.tile_pool(name="psf", bufs=2, space="PSUM") as psf,
        ):
            # ---- prologue loads.  DMA-engine order: hg0, pre, idx, hg1, aux
            hg_tiles = {}
            for g in range(NCOPY):
                w0 = gstart[g]
                gch = off[gstart[g + 1]] - off[w0]
                hg = hgp.tile([128, gch, C], f16, tag="hg")
                nc.sync.dma_start(
                    hg[:],
                    hT.ap()[off[w0] * 128:off[gstart[g + 1]] * 128]
                    .rearrange("(ch p) c -> p ch c", p=128))
                hg_tiles[g] = hg
                if g == 0:
                    pre_sb = pp.tile([128, PREW], f16)
                    nc.sync.dma_start(pre_sb[:], pre_d.ap())
                    idx_sb = pp.tile([128, IDXW], mybir.dt.int16)
                    nc.sync.dma_start(idx_sb[:], idx_d.ap())
            xr_sb = pre_sb[:, :nchunk * 8]
            dl_sb = pre_sb[:, nchunk * 8:nchunk * 9]
            iota_sb = pre_sb[:, nchunk * 9:]
            aux_sb = pp.tile([128, 2], f32)
            nc.sync.dma_start(aux_sb[:], aux_d.ap())
            bias_sb = aux_sb

            # ---- dst-slot one-hot mask, split so window 0 unblocks early
            # mask_all[p, ch, d] = (dl[p, ch] == d)   (fp16)
            mask_all = pp.tile([128, nchunk * SLOTS], f16)
            for lo, hi in ((0, pre_chunks), (pre_chunks, nchunk)):
                nc.vector.tensor_tensor(
                    mask_all[:, lo * SLOTS:hi * SLOTS].rearrange(
                        "p (ch d) -> p ch d", d=SLOTS),
                    dl_sb[:, lo:hi].unsqueeze(-1).broadcast_to(
                        [128, hi - lo, SLOTS]),
                    iota_sb.unsqueeze(1).broadcast_to([128, hi - lo, SLOTS]),
                    mybir.AluOpType.is_equal,
                )

            # res accumulator in SBUF, fp16; col = w*128 + k*16 + d
            resstack = [pp.tile([128, WINDOWS * KS], f16,
                                name=f"resstack{i}")
                        for i in range(2)]   # per c_in half

            wt_sb = pp.tile([128, 16 * C], f16)  # chunk q at cols [q*256,..)
            wt_emitted = 0

            # ---- PE warm-up: matmul activity releases the HAM clock
            # throttle while the prologue DMAs fill.  Lands in resstack[0]
            # (overwritten by copy group 0) so dead-code passes keep it.
            wu = pp.tile([128, 512], f16, name="wu")
            nc.vector.memset(wu[:], 0.0)
            wps = psf.tile([128, 512], f32, tag="wps", name="wps")
            for i in range(NWU):
                nc.tensor.matmul(wps[:], wu[:, :128], wu[:],
                                 start=(i == 0), stop=(i == NWU - 1))
            nc.scalar.copy(resstack[0][:, 0:512], wps[:])

            out_sb = [pp.tile([128, DSTS_PER_CORE], f16, name=f"osb{oh}")
                      for oh in range(2)]

            split_bounds = np.cumsum([0] + SPLITS)
            next_split = 0
            psq = {}

            def emit_final_split(s):
                w0, w1 = int(split_bounds[s]), int(split_bounds[s + 1])
                nw = w1 - w0
                for oh in range(2):
                    pso = psf.tile([128, nw * SLOTS], f32, tag="psf",
                                   name=f"pso{oh}")
                    for q in range(16):      # q = (k, ci_half)
                        k, cih = divmod(q, 2)
                        rs = resstack[cih][:]
                        rhs_ap = bass.AP(
                            rs.tensor,
                            rs.offset + w0 * KS + k * SLOTS,
                            [[WINDOWS * KS, 128], [KS, nw], [1, SLOTS]],
                        )
                        nc.tensor.matmul(
                            pso[:],
                            wt_sb[:, q * 256 + oh * 128:
                                  q * 256 + oh * 128 + 128],
                            rhs_ap,
                            start=(q == 0),
                            stop=(q == 15),
                        )
                    nc.scalar.add(
                        out_sb[oh][:, w0 * SLOTS:w1 * SLOTS],
                        pso[:], bias_sb[:, oh:oh + 1])
                # store exactly this split's columns: a store spanning two
                # adds can miss the dependency on the earlier one
                for oh in range(2):
                    nc.sync.dma_start(
                        out_d.ap()[oh * 128:(oh + 1) * 128,
                                   w0 * SLOTS:w1 * SLOTS],
                        out_sb[oh][:, w0 * SLOTS:w1 * SLOTS])

            for g, gg in enumerate(GROUPS):
                w0 = gstart[g]
                gch = off[w0 + gg] - off[w0]   # chunks in this group
                if g < NCOPY:
                    hg = hg_tiles[g]
                else:
                    hg = hgp.tile([128, gch, C], f16, tag="hg")
                    nc.gpsimd.dma_gather(
                        out_ap=hg[:],
                        in_ap=hT.ap(),
                        idxs_ap=idx_sb[:, (off[w0] - pre_chunks) * 8:
                                       (off[w0 + gg] - pre_chunks) * 8],
                        num_idxs=gch * 128,
                        num_idxs_reg=gch * 128,
                        elem_size=C,
                        # single-packet mode traps the exec unit >1024 descs
                        single_packet=False,
                    )
                # wT feeds only the final matmuls; emit its two halves
                # behind the early gathers so they land mid-stream
                if g == NCOPY + 1 and wt_emitted == 0:
                    nc.sync.dma_start(
                        wt_sb[:, :8 * C],
                        wT.ap()[:8 * 128].rearrange(
                            "(q p) c -> p q c", p=128))
                    wt_emitted = 1
                if g == NCOPY + 2 and wt_emitted == 1:
                    nc.sync.dma_start(
                        wt_sb[:, 8 * C:],
                        wT.ap()[8 * 128:].rearrange(
                            "(q p) c -> p q c", p=128))
                    wt_emitted = 2
                for wl in range(gg):
                    w = w0 + wl
                    cw = caps[w]
                    ci_ = copy_of_w[w]
                    gran = COPY_GRAN[ci_]
                    qc = w - cstart[ci_]
                    if qc == 0:
                        psq = {half: psw.tile([128, gran * KS], f32,
                                              tag=f"psq{half}",
                                              name=f"psq{half}")
                               for half in range(2)}
                    # rhs[p, ch, k, d] = xr[p, ch, k] * mask[p, ch, d]
                    rhs = rhp.tile([128, cw, KS], f16, tag="rhs")
                    xr_w = bass.AP(xr_sb.tensor,
                                   xr_sb.offset + off[w] * K,
                                   [[PREW, 128], [K, cw],
                                    [1, K], [0, SLOTS]])
                    mk_w = bass.AP(mask_all[:].tensor,
                                   mask_all[:].offset + off[w] * SLOTS,
                                   [[nchunk * SLOTS, 128], [SLOTS, cw],
                                    [0, K], [1, SLOTS]])
                    build_eng = nc.gpsimd if (w % 8) == 5 else nc.vector
                    build_eng.tensor_tensor(
                        rhs[:].rearrange("p ch (k d) -> p ch k d", d=SLOTS),
                        xr_w, mk_w, mybir.AluOpType.mult,
                    )

                    # aggregate: psq[ci, qc*KS+(k,d)] += Hg_chunk.T @ rhs
                    for half in range(2):
                        for ch in range(cw):
                            nc.tensor.matmul(
                                psq[half][:, qc * KS:(qc + 1) * KS],
                                hg[:, off[w] - off[w0] + ch,
                                   half * 128:(half + 1) * 128],
                                rhs[:, ch, :],
                                start=(ch == 0),
                                stop=(ch == cw - 1),
                            )
                    if qc == gran - 1:
                        for half in range(2):
                            nc.scalar.copy(
                                resstack[half][:, cstart[ci_] * KS:
                                               cstart[ci_ + 1] * KS],
                                psq[half][:])
                        if _os.environ.get("GNN_INTERLEAVE", "1") == "1":
                            while (next_split < len(SPLITS) and
                                   split_bounds[next_split + 1] <= w + 1):
                                emit_final_split(next_split)
                                next_split += 1
            while next_split < len(SPLITS):
                emit_final_split(next_split)
                next_split += 1

    nc.compile()
    return nc


# ------------------------------------------------------------------ host ----
def _greedy_partition(items_deg, nbins, cap):
    """Assign item ids (sorted desc by degree) to bins; each bin gets at most
    `cap` items, minimizing max degree-sum.  Returns list of lists."""
    import heapq
    bins = [[] for _ in range(nbins)]
    heap = [(0, b) for b in range(nbins)]
    heapq.heapify(heap)
    for it, dg in items_deg:
        s, b = heapq.heappop(heap)
        bins[b].append(it)
        if len(bins[b]) < cap:
            heapq.heappush(heap, (s + dg, b))
    return bins


def _pack_windows(items_deg, caps):
    """Pack (dst, deg) items into len(caps) bins of SLOTS items each with
    bin w's degree-sum <= caps[w]*128.  Returns list of lists or None."""
    nb = len(caps)
    rem_cap = [c * 128 for c in caps]
    rem_slots = [SLOTS] * nb
    bins = [[] for _ in range(nb)]
    for it, dg in items_deg:           # desc by degree
        best, best_score = -1, None
        for b in range(nb):
            if rem_slots[b] == 0 or rem_cap[b] < dg:
                continue
            score = (rem_cap[b] - dg) / rem_slots[b]
            if best_score is None or score > best_score:
                best, best_score = b, score
        if best < 0:
            return None
        bins[best].append(it)
        rem_cap[best] -= dg
        rem_slots[best] -= 1
    return bins


def _arrange_caps(capset):
    """Order a multiset of per-window capacities: small ones first (fast
    prologue copies), the rest descending in the middle, two small last
    (short tail chain)."""
    s = sorted(capset)
    front, back = s[:5], s[5:7]
    mid = sorted(s[7:], reverse=True)
    return front + mid + back


def _choose_caps(core_items):
    """Search cap profiles (nondecreasing totals) until every core packs.
    Returns (caps, core_windows)."""
    profiles = []
    for extra in range(0, 33):
        for a in range(0, min(extra // 2, 8) + 1):
            b = extra - 2 * a
            if a + b > WINDOWS:
                continue
            profiles.append((extra, a, b))
    profiles.sort()
    for extra, a, b in profiles:
        caps = _arrange_caps([6] * a + [5] * b + [4] * (WINDOWS - a - b))
        packs = []
        for items in core_items:
            p = _pack_windows(items, caps)
            if p is None:
                break
            packs.append(p)
        if len(packs) == NCORES:
            return caps, packs
    # fallback: uniform capacity from the worst window under plain LPT
    max_cnt = 0
    core_windows = []
    for items in core_items:
        degmap = dict(items)
        wins = _greedy_partition(items, WINDOWS, SLOTS)
        core_windows.append(wins)
        for wlist in wins:
            max_cnt = max(max_cnt, int(sum(degmap[m] for m in wlist)))
    caps = [max(1, -(-max_cnt // 128))] * WINDOWS
    return caps, core_windows


def kernel(h, X, edge_index, batch_node, weight, bias):
    h = np.asarray(h, dtype=np.float32)
    X = np.asarray(X, dtype=np.float32)
    edge_index = np.asarray(edge_index)
    weight = np.asarray(weight, dtype=np.float32)
    bias = np.asarray(bias, dtype=np.float32)

    src = edge_index[0].astype(np.int64)
    dst = edge_index[1].astype(np.int64)

    deg = np.bincount(dst, minlength=N)
    order = np.argsort(-deg, kind="stable")

    # dst -> core (8 bins of 512)
    core_bins = _greedy_partition([(int(m), int(deg[m])) for m in order],
                                  NCORES, DSTS_PER_CORE)
    core_items = [sorted([(m, int(deg[m])) for m in core_bins[c]],
                         key=lambda t: -t[1]) for c in range(NCORES)]
    caps, core_windows = _choose_caps(core_items)

    # edges grouped by dst
    eorder = np.argsort(dst, kind="stable")
    starts = np.searchsorted(dst[eorder], np.arange(N))
    ends = np.searchsorted(dst[eorder], np.arange(N) + 1)

    key = tuple(caps)
    if key not in _prog_cache:
        _prog_cache[key] = _build_program(caps)
    nc = _prog_cache[key]

    off = [0]
    for cw in caps:
        off.append(off[-1] + cw)
    nchunk = off[-1]
    gstart = [0]
    for g in GROUPS:
        gstart.append(gstart[-1] + g)
    wcut = gstart[NCOPY]
    pre_chunks = off[wcut]
    R0 = pre_chunks * 128
    PREW = nchunk * 9 + SLOTS
    IDXW = (nchunk - pre_chunks) * 8

    hT16 = np.ascontiguousarray(h.T).astype(np.float16)       # [N, C]
    wTh = np.ascontiguousarray(
        weight.transpose(0, 2, 1).reshape(K * C, C)).astype(np.float16)
    bias2 = np.ascontiguousarray(bias.reshape(2, 128))
    iota = np.tile(np.arange(SLOTS, dtype=np.float16), (128, 1))

    in_maps = []
    perms = []
    for c in range(NCORES):
        idx = np.zeros((128, IDXW), dtype=np.int16)
        xr = np.zeros((128, nchunk, K), dtype=np.float16)
        dl = np.zeros((128, nchunk), dtype=np.float16)
        prefix_srcs = np.zeros(R0, dtype=np.int64)  # row j of the hT prefix
        perm = np.empty(DSTS_PER_CORE, dtype=np.int64)
        for w in range(WINDOWS):
            wl = core_windows[c][w]
            el = []
            sl = []
            for d_slot, m in enumerate(wl):
                perm[w * SLOTS + d_slot] = m
                ee = eorder[starts[m]:ends[m]]
                el.append(ee)
                sl.append(np.full(ee.shape[0], d_slot, dtype=np.float16))
            el = (np.concatenate(el) if el else
                  np.empty(0, dtype=np.int64))
            sl = (np.concatenate(sl) if sl else
                  np.empty(0, dtype=np.float16))
            # order the window's edges by src: the dst slot travels in `sl`,
            # and src-sorted gathers hit HBM row buffers far more often
            so = np.argsort(src[el], kind="stable")
            el, sl = el[so], sl[so]
            L = el.shape[0]
            j = np.arange(L)
            p = j % 128
            ch = off[w] + j // 128
            xr[p, ch, :] = X[el, :].astype(np.float16)
            dl[p, ch] = sl
            if w < wcut:
                # position j of the window -> row off[w]*128 + j of prefix
                prefix_srcs[off[w] * 128 + j] = src[el]
            else:
                # gather index layout: pos j -> [j%16, j//16], replicated x8
                srcs = (src[el] + R0).astype(np.int16)
                blk = np.zeros((16, caps[w] * 8), dtype=np.int16)
                blk[j % 16, j // 16] = srcs
                icol = (off[w] - pre_chunks) * 8
                idx[:, icol:icol + caps[w] * 8] = np.tile(blk, (8, 1))
        hT2 = np.concatenate([hT16[prefix_srcs], hT16], axis=0)
        pre = np.concatenate(
            [xr.reshape(128, nchunk * K), dl, iota], axis=1)
        assert pre.shape[1] == PREW
        in_maps.append({
            "hT": np.ascontiguousarray(hT2), "wT": wTh,
            "aux": np.ascontiguousarray(bias2.T.astype(np.float32)),
            "pre": np.ascontiguousarray(pre),
            "idx": np.ascontiguousarray(idx),
        })
        perms.append(perm)

    global _last_perms
    _last_perms = perms

    try:
        res = run_bass_kernel_spmd(nc, in_maps, core_ids=list(range(NCORES)))
    except Exception:
        # transient device-state issues (e.g. a previous crashed process left
        # a core unrecoverable) usually clear on retry
        time.sleep(10)
        res = run_bass_kernel_spmd(nc, in_maps, core_ids=list(range(NCORES)))

    out = np.empty((C, N), dtype=np.float32)
    for c in range(NCORES):
        out[:, perms[c]] = res.results[c]["out"].astype(np.float32)
    return out
